# revision 26
# baseline (speedup 1.0000x reference)
"""Trainium2 Bass kernel for CustomStellarModel2 (GNN message passing), v2.

Self-contained: host-side sharding/preprocessing + Bass/Tile kernel
compiled and run on 8 NeuronCores via PJRT (axon), then unsharded.

Strategy (v2, "transposed two-level gather"):
  - Nodes sharded contiguously across 8 cores (12500/core, padded to
    12544). Within a core, nodes are sorted by local in-degree
    (descending) and grouped into G=98 groups of 128 dst nodes.
  - Edge slots per group laid out dst-major: slot = d*J_g + j where
    J_g = max degree in group (shared across cores). Degree sorting
    keeps padding ~2%.
  - Level-1 gather: per block of BLK groups, the distinct source rows
    (bank-windowed so indices fit int16) are gathered from the
    AllGather'd node tables (xl / q|v) into a compact SBUF table.
    Entry 0 of each block table is reserved and zeroed (pad target).
  - Level-2 gather: SBUF-source dma_gather (transpose=True) expands the
    compact table into TRANSPOSED [feature, slot] layout.
  - gamma/beta (and k in layer 2) are per-dst and applied via stride-0
    broadcast APs on DVE; the segment sum is a contiguous
    tensor_reduce over the inner j axis. No one-hot matrices, no PSUM
    in the edge phases.
  - Pad slots gather the zero entry; layer-1 pad contribution
    m_d*relu(beta') is subtracted exactly (correction trick); layer-2
    pads contribute sigmoid(k)*0 = 0.
  - Mean aggregation folds 1/cnt into gamma/beta via relu positive
    homogeneity: relu(g*x+b)/c = relu((g/c)*x + (b/c)).
  - Two AllGather collectives replicate the gather tables (xl, qv).
"""
import math
import numpy as np
import ml_dtypes

BF16 = ml_dtypes.bfloat16

IN_DIM, H, C_OUT = 64, 128, 20
N_CORES = 8


def _config(n=100000, e=1600000, bank=32768, cap=7168, gmax=1024,
            gmax2=896, scratch=65536):
    global N, E, NC, G, NP, NFULL, BANK, N_BANKS, CAP, GMAX, GMAX2, SCRATCH
    N, E, BANK, CAP, GMAX, GMAX2, SCRATCH = n, e, bank, cap, gmax, gmax2, \
        scratch
    NC = N // N_CORES
    G = math.ceil(NC / 128)
    NP = G * 128
    NFULL = NP * N_CORES
    N_BANKS = math.ceil(NFULL / BANK)


_config()
ABLATE = set()


# ---------------------------------------------------------------------------
# Host-side preprocessing
# ---------------------------------------------------------------------------

def _wrap16(idx):
    """[128, n/16] int16 blob: wrapped into 16 partitions, replicated 8x."""
    n = len(idx)
    assert n % 16 == 0
    blk = np.zeros((16, n // 16), np.int16)
    ii = np.arange(n)
    blk[ii % 16, ii // 16] = idx.astype(np.int16)
    return np.tile(blk, (8, 1))


def _prep_edges(edge_index):
    src = np.asarray(edge_index[0]).astype(np.int64)
    dst = np.asarray(edge_index[1]).astype(np.int64)
    core = dst // NC

    perms, degs, edata = [], [], []
    rowof = np.empty(N, np.int64)
    for c in range(N_CORES):
        m = core == c
        dl = dst[m] - c * NC
        deg = np.bincount(dl, minlength=NC)
        perm = np.argsort(-deg, kind="stable")
        inv = np.empty(NC, np.int64)
        inv[perm] = np.arange(NC)
        rowof[c * NC:(c + 1) * NC] = c * NP + inv
        perms.append(perm)
        ds = np.zeros(NP, np.int64)
        ds[:NC] = deg[perm]
        degs.append(ds)
        edata.append((m, dl, inv))

    percore = []
    J = np.zeros((N_CORES, G), np.int64)
    for c in range(N_CORES):
        m, dl, inv = edata[c]
        d_s = inv[dl]
        s_r = rowof[src[m]]
        order = np.lexsort((s_r, d_s))
        percore.append((d_s[order], s_r[order]))
        J[c] = degs[c].reshape(G, 128).max(axis=1)
    Jg = J.max(axis=0)
    g_soff = np.zeros(G + 1, np.int64)
    g_soff[1:] = np.cumsum(128 * Jg)
    S_TOT = int(g_soff[-1])

    # per-core unique (group, src-row) entries
    peruniq = []
    Dg = np.zeros((N_CORES, G), np.int64)   # distinct srcs per (core, group)
    for c in range(N_CORES):
        ed, es = percore[c]
        key = (ed // 128) * NFULL + es
        uk = np.unique(key)
        ug, ur = uk // NFULL, uk % NFULL
        ub = ur // BANK
        Dg[c] += np.bincount(ug, minlength=G)
        peruniq.append((uk, ug, ub))

    # greedy variable-size blocks: max-over-cores entries + padding <= CAP
    Dgm = Dg.max(axis=0)
    bounds = [0]
    acc = 0
    for g in range(G):
        if acc > 0 and acc + Dgm[g] + 1 + N_BANKS * 127 > CAP:
            bounds.append(g)
            acc = 0
        acc += Dgm[g]
    bounds.append(G)
    N_BLK = len(bounds) - 1
    blk_of_g = np.zeros(G, np.int64)
    for b in range(N_BLK):
        blk_of_g[bounds[b]:bounds[b + 1]] = b

    runs = np.zeros((N_CORES, N_BLK, N_BANKS), np.int64)
    for c in range(N_CORES):
        uk, ug, ub = peruniq[c]
        ublk = blk_of_g[ug]
        cnt = np.bincount(ublk * N_BANKS + ub, minlength=N_BLK * N_BANKS)
        runs[c] = cnt.reshape(N_BLK, N_BANKS)

    B = runs.max(axis=0)
    B[:, 0] += 1                      # reserved zero entry per block
    Bpad = ((B + 127) // 128) * 128
    bank_off = np.zeros((N_BLK, N_BANKS + 1), np.int64)
    bank_off[:, 1:] = np.cumsum(Bpad, axis=1)
    blk_entries = Bpad.sum(axis=1)          # entries per block table
    assert blk_entries.max() <= 32767
    l1_seg = np.zeros(N_BLK * N_BANKS + 1, np.int64)
    l1_seg[1:] = np.cumsum(Bpad.reshape(-1))
    L1_TOT = int(l1_seg[-1])

    core_blobs = []
    for c in range(N_CORES):
        uk, ug, ub = peruniq[c]
        ublk = blk_of_g[ug]
        o2 = np.lexsort((uk, ub, ublk))
        seg = ublk[o2] * N_BANKS + ub[o2]
        seg_start = np.searchsorted(seg, np.arange(N_BLK * N_BANKS))
        rank = np.arange(len(seg)) - seg_start[seg]
        isb0 = (ub[o2] == 0).astype(np.int64)
        pos_in_blk = bank_off[ublk[o2], ub[o2]] + rank + isb0
        pos_by_uk = np.empty(len(uk), np.int64)
        pos_by_uk[o2] = pos_in_blk
        # L1 idx blob (bank-relative rows)
        l1_idx = np.zeros(L1_TOT, np.int64)
        l1_idx[l1_seg[seg] + rank + isb0] = (uk[o2] % NFULL) - ub[o2] * BANK
        # L2 idx blob (block-table entry per slot)
        ed, es = percore[c]
        eidx = np.searchsorted(uk, (ed // 128) * NFULL + es)
        epos = pos_by_uk[eidx]
        starts = np.searchsorted(ed, np.arange(NP))
        erank = np.arange(len(ed)) - starts[ed]
        g_e = ed // 128
        slot = g_soff[g_e] + (ed % 128) * Jg[g_e] + erank
        l2_idx = np.zeros(S_TOT, np.int64)
        l2_idx[slot] = epos
        # scale blobs
        inv_r = (1.0 / np.maximum(degs[c], 1)).astype(np.float32)
        mneg = -(np.repeat(Jg, 128) - degs[c]).astype(np.float32)
        core_blobs.append(dict(
            l1i=_wrap16(l1_idx), l2i=_wrap16(l2_idx),
            invT=np.tile(inv_r[None, :], (128, 1)),
            mT=np.tile(mneg[None, :].astype(BF16), (128, 1)),
        ))

    meta = dict(Jg=Jg.astype(int), g_soff=g_soff, S_TOT=S_TOT,
                Bpad=Bpad, bank_off=bank_off, l1_seg=l1_seg, L1_TOT=L1_TOT,
                blk_entries=blk_entries.astype(int), perms=perms,
                bounds=bounds)
    return meta, core_blobs


def _prep_weights(inp):
    f = lambda a: np.ascontiguousarray(a, dtype=np.float32)
    W1T = f(inp["W1"]).T.astype(BF16)                      # [64,128]
    b1r = f(inp["b1"])[None, :].astype(BF16)               # [1,128]
    WlT = f(inp["Wl"]).T.astype(BF16)                      # [128,128]
    # transposed-layer weight slices: gamma, beta, gamma_s, beta_s, xls
    WfT, WfsT = f(inp["Wf"]).T, f(inp["Wfs"]).T
    R1T = np.concatenate(
        [WfT[:, H:], WfT[:, :H], WfsT[:, H:], WfsT[:, :H],
         f(inp["Wls"]).T], axis=1).astype(BF16)            # [128, 640]
    bias1T = np.concatenate(
        [f(inp["bf"])[H:], f(inp["bf"])[:H], f(inp["bfs"])[H:],
         f(inp["bfs"])[:H], np.zeros(H, np.float32)])[None, :].astype(BF16)
    R2T = np.concatenate([f(inp["Wk"]).T, f(inp["Wskip"]).T],
                         axis=1).astype(BF16)              # [128, 256]
    bias2T = np.concatenate([f(inp["bk"]), f(inp["bres"])])[None, :].astype(BF16)
    R2R = np.concatenate([f(inp["Wq"]).T, f(inp["Wv"]).T],
                         axis=1).astype(BF16)              # [128, 256]
    bias2R = np.concatenate([f(inp["bq"]), f(inp["bv"])])[None, :].astype(BF16)
    WfcT = f(inp["Wfc"]).T.astype(BF16)                    # [128, 20]
    bfcR = f(inp["bfc"])[None, :].astype(BF16)             # [1, 20]
    return dict(W1T=W1T, b1r=b1r, WlT=WlT, R1T=R1T, bias1T=bias1T,
                R2T=R2T, bias2T=bias2T, R2R=R2R, bias2R=bias2R,
                WfcT=WfcT, bfcR=bfcR)


def _make_in_maps(x, meta, core_blobs, w):
    in_maps = []
    for c in range(N_CORES):
        perm = meta["perms"][c]
        xT_c = np.zeros((IN_DIM, NP), BF16)
        xT_c[:, :NC] = x[c * NC:(c + 1) * NC][perm].T.astype(BF16)
        mp = dict(xT=xT_c, **{k: w[k] for k in w})
        mp.update(core_blobs[c])
        in_maps.append(mp)
    return in_maps


def _unpermute(results, meta):
    logits = np.empty((N, C_OUT), np.float32)
    for c in range(N_CORES):
        out_sorted = results[c]["outT"][:, :NC].T
        loc = logits[c * NC:(c + 1) * NC]
        loc[meta["perms"][c]] = out_sorted
    return logits


# ---------------------------------------------------------------------------
# Bass kernel builder
# ---------------------------------------------------------------------------

def _build(meta):
    import concourse.bass as bass
    import concourse.bacc as bacc
    import concourse.mybir as mybir
    import concourse.tile as tile
    from concourse import library_config

    dt = mybir.dt
    Jg = meta["Jg"]
    g_soff = meta["g_soff"]
    S_TOT = meta["S_TOT"]
    Bpad = meta["Bpad"]
    bank_off = meta["bank_off"]
    l1_seg = meta["l1_seg"]
    L1_TOT = meta["L1_TOT"]
    blk_entries = meta["blk_entries"]
    bounds = meta["bounds"]
    N_BLK = len(bounds) - 1
    SMAX = int(128 * max(Jg))                 # max slots per group
    RMAX = int(max(blk_entries)) // 128       # max block-table slabs
    L1IMAX = int(Bpad.sum(axis=1).max()) // 16

    nc = bacc.Bacc("TRN2", target_bir_lowering=False, debug=False,
                   num_devices=N_CORES, dynamic_dma_scratch_size=SCRATCH,
                   num_swdge_queues=4)
    qctr = iter(range(10 ** 9))

    # ---- external inputs ----
    xT = nc.dram_tensor("xT", [IN_DIM, NP], dt.bfloat16,
                        kind="ExternalInput").ap()
    W1T = nc.dram_tensor("W1T", [IN_DIM, H], dt.bfloat16,
                         kind="ExternalInput").ap()
    b1r = nc.dram_tensor("b1r", [1, H], dt.bfloat16, kind="ExternalInput").ap()
    WlT = nc.dram_tensor("WlT", [H, H], dt.bfloat16, kind="ExternalInput").ap()
    R1T = nc.dram_tensor("R1T", [H, 5 * H], dt.bfloat16,
                         kind="ExternalInput").ap()
    bias1T = nc.dram_tensor("bias1T", [1, 5 * H], dt.bfloat16,
                            kind="ExternalInput").ap()
    R2T = nc.dram_tensor("R2T", [H, 2 * H], dt.bfloat16,
                         kind="ExternalInput").ap()
    bias2T = nc.dram_tensor("bias2T", [1, 2 * H], dt.bfloat16,
                            kind="ExternalInput").ap()
    R2R = nc.dram_tensor("R2R", [H, 2 * H], dt.bfloat16,
                         kind="ExternalInput").ap()
    bias2R = nc.dram_tensor("bias2R", [1, 2 * H], dt.bfloat16,
                            kind="ExternalInput").ap()
    WfcT = nc.dram_tensor("WfcT", [H, C_OUT], dt.bfloat16,
                          kind="ExternalInput").ap()
    bfcR = nc.dram_tensor("bfcR", [1, C_OUT], dt.bfloat16,
                          kind="ExternalInput").ap()
    l1i_d = nc.dram_tensor("l1i", [128, L1_TOT // 16], dt.int16,
                           kind="ExternalInput").ap()
    l2i_d = nc.dram_tensor("l2i", [128, S_TOT // 16], dt.int16,
                           kind="ExternalInput").ap()
    invT_d = nc.dram_tensor("invT", [128, NP], dt.float32,
                            kind="ExternalInput").ap()
    mT_d = nc.dram_tensor("mT", [128, NP], dt.bfloat16,
                          kind="ExternalInput").ap()
    outT = nc.dram_tensor("outT", [C_OUT, NP], dt.float32,
                          kind="ExternalOutput").ap()

    with tile.TileContext(nc) as tc:
        with (
            tc.tile_pool(name="dram", bufs=1, space="DRAM") as dp,
            tc.tile_pool(name="const", bufs=1) as cp,
        ):
            nc.gpsimd.load_library(library_config.mlp)
            # DRAM intermediates
            xl_loc = dp.tile([NP, H], dt.bfloat16)
            xl_full = dp.tile([NFULL, H], dt.bfloat16)
            gbT_loc = dp.tile([G, 128, 2 * H], dt.bfloat16)
            skipT_loc = dp.tile([G, 128, H], dt.bfloat16)
            kT_loc = dp.tile([G, 128, H], dt.bfloat16)
            skip2T_loc = dp.tile([G, 128, H], dt.bfloat16)
            qv_loc = dp.tile([NP, 2 * H], dt.bfloat16)
            qv_full = dp.tile([NFULL, 2 * H], dt.bfloat16)

            # constants
            ones = cp.tile([1, 128], dt.bfloat16)
            nc.vector.memset(ones[:], 1.0)
            w1t_t = cp.tile([IN_DIM, H], dt.bfloat16)
            nc.sync.dma_start(out=w1t_t[:], in_=W1T[:])
            b1r_t = cp.tile([1, H], dt.bfloat16)
            nc.sync.dma_start(out=b1r_t[:], in_=b1r[:])
            wlt_t = cp.tile([H, H], dt.bfloat16)
            nc.sync.dma_start(out=wlt_t[:], in_=WlT[:])
            r1t_t = cp.tile([H, 5 * H], dt.bfloat16)
            nc.sync.dma_start(out=r1t_t[:], in_=R1T[:])
            bias1t_t = cp.tile([1, 5 * H], dt.bfloat16)
            nc.sync.dma_start(out=bias1t_t[:], in_=bias1T[:])
            r2t_t = cp.tile([H, 2 * H], dt.bfloat16)
            nc.sync.dma_start(out=r2t_t[:], in_=R2T[:])
            bias2t_t = cp.tile([1, 2 * H], dt.bfloat16)
            nc.sync.dma_start(out=bias2t_t[:], in_=bias2T[:])
            r2r_t = cp.tile([H, 2 * H], dt.bfloat16)
            nc.sync.dma_start(out=r2r_t[:], in_=R2R[:])
            bias2r_t = cp.tile([1, 2 * H], dt.bfloat16)
            nc.sync.dma_start(out=bias2r_t[:], in_=bias2R[:])
            wfc_t = cp.tile([H, C_OUT], dt.bfloat16)
            nc.sync.dma_start(out=wfc_t[:], in_=WfcT[:])
            bfc_t = cp.tile([1, C_OUT], dt.bfloat16)
            nc.sync.dma_start(out=bfc_t[:], in_=bfcR[:])

            relu = mybir.ActivationFunctionType.Relu
            sigm = mybir.ActivationFunctionType.Sigmoid

            # ---------------- P1: dense layer 1 ---------------------------
            with (
                tc.tile_pool(name="p1sb", bufs=3) as sb,
                tc.tile_pool(name="p1ps", bufs=2, space="PSUM") as ps1,
                tc.tile_pool(name="p1pt", bufs=2, space="PSUM") as pst,
                tc.tile_pool(name="p1px", bufs=2, space="PSUM") as psx,
            ):
                for g in range(G):
                    c0 = g * 128
                    xt_t = sb.tile([IN_DIM, 128], dt.bfloat16, tag="xt")
                    nc.sync.dma_start(out=xt_t[:], in_=xT[:, c0:c0 + 128])
                    p1 = ps1.tile([128, 128], dt.float32, tag="p1")
                    nc.tensor.matmul(p1[:], lhsT=w1t_t[:], rhs=xt_t[:],
                                     start=True, stop=False)
                    nc.tensor.matmul(p1[:], lhsT=b1r_t[:], rhs=ones[:],
                                     start=False, stop=True)
                    x1t = sb.tile([128, 128], dt.bfloat16, tag="x1t")
                    nc.scalar.activation(x1t[:], p1[:], relu)
                    # xl rows (for AllGather table)
                    pxl = psx.tile([128, H], dt.float32, tag="pxl")
                    nc.tensor.matmul(pxl[:], lhsT=x1t[:], rhs=wlt_t[:],
                                     start=True, stop=True)
                    xlr = sb.tile([128, H], dt.bfloat16, tag="xlr")
                    nc.scalar.copy(xlr[:], pxl[:])
                    nc.sync.dma_start(out=xl_loc[c0:c0 + 128, :], in_=xlr[:])
                    # transposed streams: gamma, beta, gamma_s, beta_s, xls
                    pT = pst.tile([128, 5, 128], dt.float32, tag="pT")
                    for i in range(5):
                        nc.tensor.matmul(pT[:, i, :],
                                         lhsT=r1t_t[:, i * H:(i + 1) * H],
                                         rhs=x1t[:], start=True, stop=False)
                        nc.tensor.matmul(pT[:, i, :],
                                         lhsT=bias1t_t[:, i * H:(i + 1) * H],
                                         rhs=ones[:], start=False, stop=True)
                    # gamma' / beta' = (gamma|beta) * inv
                    invc = sb.tile([128, 128], dt.float32, tag="invc")
                    nc.sync.dma_start(out=invc[:], in_=invT_d[:, c0:c0 + 128])
                    gbT = sb.tile([128, 2 * H], dt.bfloat16, tag="gbT")
                    nc.vector.tensor_mul(gbT[:, :H], pT[:, 0, :], invc[:])
                    nc.vector.tensor_mul(gbT[:, H:], pT[:, 1, :], invc[:])
                    nc.sync.dma_start(out=gbT_loc[g, :, :], in_=gbT[:])
                    # FiLM skip: relu(gs*xls + bs), transposed
                    sks = sb.tile([128, 3, 128], dt.bfloat16, tag="sks")
                    nc.scalar.copy(sks[:], pT[:, 2:5, :])
                    t0 = sb.tile([128, 128], dt.bfloat16, tag="t0")
                    nc.vector.tensor_mul(t0[:], sks[:, 0, :], sks[:, 2, :])
                    t1 = sb.tile([128, 128], dt.bfloat16, tag="t1")
                    nc.vector.tensor_add(t1[:], t0[:], sks[:, 1, :])
                    skT = sb.tile([128, 128], dt.bfloat16, tag="skT")
                    nc.scalar.activation(skT[:], t1[:], relu)
                    nc.sync.dma_start(out=skipT_loc[g, :, :], in_=skT[:])

            # ---------------- P2: AllGather xl ----------------------------
            nc.gpsimd.collective_compute(
                "AllGather", mybir.AluOpType.bypass,
                replica_groups=[list(range(N_CORES))],
                ins=[xl_loc[:]], outs=[xl_full[:]],
            )

            # ---------------- P3+P4: FiLM edge phase + dense 2 ------------
            with (
                tc.tile_pool(name="p3tb", bufs=2) as tbp,
                tc.tile_pool(name="p3sb", bufs=2) as sb,
                tc.tile_pool(name="p3g", bufs=2) as gp,
                tc.tile_pool(name="p3ps", bufs=2, space="PSUM") as psk,
                tc.tile_pool(name="p3pq", bufs=2, space="PSUM") as psq,
            ):
                for blk in range(N_BLK):
                    glo, ghi = bounds[blk], bounds[blk + 1]
                    nslab = int(blk_entries[blk]) // 128
                    tbl = tbp.tile([128, RMAX, H], dt.bfloat16, tag="tbl")
                    nb16 = int(Bpad[blk].sum()) // 16
                    l1i = tbp.tile([128, L1IMAX], dt.int16, tag="l1i")
                    nc.sync.dma_start(
                        out=l1i[:, :nb16],
                        in_=l1i_d[:, l1_seg[blk * N_BANKS] // 16:
                                  l1_seg[(blk + 1) * N_BANKS] // 16])
                    iof = 0
                    if "l1" in ABLATE:
                        nc.vector.memset(tbl[:], 0.0)
                    for b in (() if "l1" in ABLATE else range(N_BANKS)):
                        ntot = int(Bpad[blk, b])
                        boff0 = int(bank_off[blk, b]) // 128
                        rows = min(BANK, NFULL - b * BANK)
                        for i0 in range(0, ntot, GMAX):
                            nidx = min(GMAX, ntot - i0)
                            boff = boff0 + i0 // 128
                            nc.gpsimd.dma_gather(
                                tbl[:, boff:boff + nidx // 128, :],
                                xl_full[b * BANK:b * BANK + rows, :],
                                l1i[:, (iof + i0) // 16:
                                    (iof + i0 + nidx) // 16],
                                nidx, nidx, H, queue_num=next(qctr) % 4)
                        iof += ntot
                    if "l1" not in ABLATE:
                        nc.vector.memset(tbl[0:1, 0, 0:H], 0.0)
                    for g in range(glo, ghi):
                        c0 = g * 128
                        J = int(Jg[g])
                        Sg = 128 * J
                        gbT = sb.tile([128, 2 * H], dt.bfloat16, tag="gbT")
                        nc.sync.dma_start(out=gbT[:], in_=gbT_loc[g, :, :])
                        skT = sb.tile([128, H], dt.bfloat16, tag="skT")
                        nc.sync.dma_start(out=skT[:], in_=skipT_loc[g, :, :])
                        mT = sb.tile([128, 128], dt.bfloat16, tag="mT")
                        nc.sync.dma_start(out=mT[:], in_=mT_d[:, c0:c0 + 128])
                        agg = sb.tile([128, 128], dt.float32, tag="agg")
                        if J > 0:
                            l2i = sb.tile([128, SMAX // 16], dt.int16,
                                          tag="l2i")
                            so = int(g_soff[g])
                            nc.sync.dma_start(
                                out=l2i[:, :Sg // 16],
                                in_=l2i_d[:, so // 16:(so + Sg) // 16])
                            xls = gp.tile([128, SMAX], dt.bfloat16, tag="xls")
                            if "l2" in ABLATE:
                                nc.vector.memset(xls[:, :Sg], 0.0)
                            for s0 in (() if "l2" in ABLATE else
                                       range(0, Sg, GMAX2)):
                                sn = min(GMAX2, Sg - s0)
                                nc.gpsimd.dma_gather(
                                    xls[:, s0:s0 + sn].unsqueeze(1),
                                    tbl[:, :nslab, :],
                                    l2i[:, s0 // 16:(s0 + sn) // 16],
                                    sn, sn, H, transpose=True,
                                    queue_num=next(qctr) % 4,
                                    sbuf_tokens_per_rank=128,
                                    sbuf_free_dim_per_rank=2 * H)
                            # msg = relu(gamma'*xl + beta'), slots [d, j]
                            A = gp.tile([128, SMAX], dt.bfloat16, tag="A")
                            Bt = gp.tile([128, SMAX], dt.bfloat16, tag="B")
                            gb_b = (gbT[:, :H].unsqueeze(2)
                                    .broadcast_to((128, H, J)))
                            be_b = (gbT[:, H:].unsqueeze(2)
                                    .broadcast_to((128, H, J)))
                            nc.vector.tensor_mul(A[:, :Sg], xls[:, :Sg], gb_b)
                            nc.vector.tensor_add(Bt[:, :Sg], A[:, :Sg], be_b)
                            nc.scalar.activation(A[:, :Sg], Bt[:, :Sg], relu)
                            nc.vector.tensor_reduce(
                                agg[:], A[:, :Sg].rearrange(
                                    "p (d j) -> p d j", j=J),
                                mybir.AxisListType.X, mybir.AluOpType.add)
                        else:
                            nc.vector.memset(agg[:], 0.0)
                        # correction: agg += mT_neg * relu(beta')
                        rb = sb.tile([128, 128], dt.bfloat16, tag="rb")
                        nc.scalar.activation(rb[:], gbT[:, H:], relu)
                        corr = sb.tile([128, 128], dt.float32, tag="corr")
                        nc.vector.tensor_mul(corr[:], rb[:], mT[:])
                        ag2 = sb.tile([128, 128], dt.float32, tag="ag2")
                        nc.vector.tensor_add(ag2[:], agg[:], corr[:])
                        x2p = sb.tile([128, 128], dt.float32, tag="x2p")
                        nc.vector.tensor_add(x2p[:], ag2[:], skT[:])
                        x2t = sb.tile([128, 128], dt.bfloat16, tag="x2t")
                        nc.scalar.activation(x2t[:], x2p[:], relu)
                        # ---- P4 dense-2 fused ----
                        pk = psk.tile([128, 2, H], dt.float32, tag="pk")
                        for i in range(2):
                            nc.tensor.matmul(pk[:, i, :],
                                             lhsT=r2t_t[:, i * H:(i + 1) * H],
                                             rhs=x2t[:],
                                             start=True, stop=False)
                            nc.tensor.matmul(
                                pk[:, i, :],
                                lhsT=bias2t_t[:, i * H:(i + 1) * H],
                                rhs=ones[:], start=False, stop=True)
                        kk = sb.tile([128, 2, H], dt.bfloat16, tag="kk")
                        nc.scalar.copy(kk[:], pk[:])
                        nc.sync.dma_start(out=kT_loc[g, :, :],
                                          in_=kk[:, 0, :])
                        nc.sync.dma_start(out=skip2T_loc[g, :, :],
                                          in_=kk[:, 1, :])
                        pq = psq.tile([128, 2 * H], dt.float32, tag="pq")
                        nc.tensor.matmul(pq[:], lhsT=x2t[:], rhs=r2r_t[:],
                                         start=True, stop=False)
                        nc.tensor.matmul(pq[:], lhsT=ones[:], rhs=bias2r_t[:],
                                         start=False, stop=True)
                        qv = sb.tile([128, 2 * H], dt.bfloat16, tag="qv")
                        nc.scalar.copy(qv[:], pq[:])
                        nc.sync.dma_start(out=qv_loc[c0:c0 + 128, :],
                                          in_=qv[:])

            # ---------------- P5: AllGather qv ----------------------------
            nc.gpsimd.collective_compute(
                "AllGather", mybir.AluOpType.bypass,
                replica_groups=[list(range(N_CORES))],
                ins=[qv_loc[:]], outs=[qv_full[:]],
            )

            # ---------------- P6: ResGated edge phase ---------------------
            with (
                tc.tile_pool(name="p6tb", bufs=2) as tbp,
                tc.tile_pool(name="p6sb", bufs=2) as sb,
                tc.tile_pool(name="p6g", bufs=2) as gp,
                tc.tile_pool(name="p6ps", bufs=2, space="PSUM") as psl,
            ):
                for blk in range(N_BLK):
                    glo, ghi = bounds[blk], bounds[blk + 1]
                    nslab = int(blk_entries[blk]) // 128
                    tbl = tbp.tile([128, RMAX, 2 * H], dt.bfloat16, tag="tbl")
                    nb16 = int(Bpad[blk].sum()) // 16
                    l1i = tbp.tile([128, L1IMAX], dt.int16, tag="l1i")
                    nc.sync.dma_start(
                        out=l1i[:, :nb16],
                        in_=l1i_d[:, l1_seg[blk * N_BANKS] // 16:
                                  l1_seg[(blk + 1) * N_BANKS] // 16])
                    iof = 0
                    if "l1" in ABLATE:
                        nc.vector.memset(tbl[:], 0.0)
                    for b in (() if "l1" in ABLATE else range(N_BANKS)):
                        ntot = int(Bpad[blk, b])
                        boff0 = int(bank_off[blk, b]) // 128
                        rows = min(BANK, NFULL - b * BANK)
                        for i0 in range(0, ntot, GMAX):
                            nidx = min(GMAX, ntot - i0)
                            boff = boff0 + i0 // 128
                            nc.gpsimd.dma_gather(
                                tbl[:, boff:boff + nidx // 128, :],
                                qv_full[b * BANK:b * BANK + rows, :],
                                l1i[:, (iof + i0) // 16:
                                    (iof + i0 + nidx) // 16],
                                nidx, nidx, 2 * H, queue_num=next(qctr) % 4)
                        iof += ntot
                    if "l1" not in ABLATE:
                        nc.vector.memset(tbl[0:1, 0, 0:2 * H], 0.0)
                    for g in range(glo, ghi):
                        J = int(Jg[g])
                        Sg = 128 * J
                        kT = sb.tile([128, H], dt.bfloat16, tag="kT")
                        nc.sync.dma_start(out=kT[:], in_=kT_loc[g, :, :])
                        sk2 = sb.tile([128, H], dt.bfloat16, tag="sk2")
                        nc.sync.dma_start(out=sk2[:], in_=skip2T_loc[g, :, :])
                        s2 = sb.tile([128, 128], dt.float32, tag="s2")
                        if J > 0:
                            l2i = sb.tile([128, SMAX // 16], dt.int16,
                                          tag="l2i")
                            so = int(g_soff[g])
                            nc.sync.dma_start(
                                out=l2i[:, :Sg // 16],
                                in_=l2i_d[:, so // 16:(so + Sg) // 16])
                            qT = gp.tile([128, SMAX], dt.bfloat16, tag="qT")
                            vT = gp.tile([128, SMAX], dt.bfloat16, tag="vT")
                            if "l2" in ABLATE:
                                nc.vector.memset(qT[:, :Sg], 0.0)
                                nc.vector.memset(vT[:, :Sg], 0.0)
                            for s0 in (() if "l2" in ABLATE else
                                       range(0, Sg, GMAX2)):
                                sn = min(GMAX2, Sg - s0)
                                nc.gpsimd.dma_gather(
                                    qT[:, s0:s0 + sn].unsqueeze(1),
                                    tbl[:, :nslab, :],
                                    l2i[:, s0 // 16:(s0 + sn) // 16],
                                    sn, sn, H, transpose=True,
                                    queue_num=next(qctr) % 4,
                                    sbuf_tokens_per_rank=128,
                                    sbuf_free_dim_per_rank=4 * H)
                            for s0 in (() if "l2" in ABLATE else
                                       range(0, Sg, GMAX2)):
                                sn = min(GMAX2, Sg - s0)
                                nc.gpsimd.dma_gather(
                                    vT[:, s0:s0 + sn].unsqueeze(1),
                                    tbl[:, :nslab, :],
                                    l2i[:, s0 // 16:(s0 + sn) // 16],
                                    sn, sn, H, transpose=True,
                                    queue_num=next(qctr) % 4,
                                    sbuf_tokens_per_rank=128,
                                    sbuf_free_dim_per_rank=4 * H,
                                    sbuf_byte_offset=2 * H)
                            A = gp.tile([128, SMAX], dt.bfloat16, tag="A")
                            Bt = gp.tile([128, SMAX], dt.bfloat16, tag="B")
                            kT_b = (kT[:].unsqueeze(2)
                                    .broadcast_to((128, H, J)))
                            nc.vector.tensor_add(A[:, :Sg], qT[:, :Sg],
                                                 kT_b)
                            nc.scalar.activation(Bt[:, :Sg], A[:, :Sg], sigm)
                            nc.vector.tensor_mul(A[:, :Sg], Bt[:, :Sg],
                                                 vT[:, :Sg])
                            nc.vector.tensor_reduce(
                                s2[:], A[:, :Sg].rearrange(
                                    "p (d j) -> p d j", j=J),
                                mybir.AxisListType.X, mybir.AluOpType.add)
                        else:
                            nc.vector.memset(s2[:], 0.0)
                        x3p = sb.tile([128, 128], dt.float32, tag="x3p")
                        nc.vector.tensor_add(x3p[:], s2[:], sk2[:])
                        x3t = sb.tile([128, 128], dt.bfloat16, tag="x3t")
                        nc.scalar.activation(x3t[:], x3p[:], relu)
                        pl = psl.tile([C_OUT, 128], dt.float32, tag="pl")
                        nc.tensor.matmul(pl[:], lhsT=wfc_t[:], rhs=x3t[:],
                                         start=True, stop=False)
                        nc.tensor.matmul(pl[:], lhsT=bfc_t[:], rhs=ones[:],
                                         start=False, stop=True)
                        lt = sb.tile([C_OUT, 128], dt.float32, tag="lt")
                        nc.scalar.copy(lt[:], pl[:])
                        nc.sync.dma_start(
                            out=outT[:, g * 128:(g + 1) * 128], in_=lt[:])

    # Align each SWDGE gather's queue with its tile-assigned DMASW lane
    # (lane = bass_scheduled_proc, assigned in scheduled order; the sim and
    # runtime lock each DMASW semaphore to a single queue).
    import os as _os
    _nq = int(_os.environ.get("NQUEUES", 1))
    from concourse.tile_sem_assignment import PROC_NAME_TO_IDX
    lane_of_proc = {PROC_NAME_TO_IDX[f"DMASW{i}"]: i for i in range(8)}
    for fn in nc.m.functions:
        for bb in fn.blocks:
            for inst in bb.instructions:
                if isinstance(inst, mybir.InstDMAGatherAnt):
                    lane = lane_of_proc.get(inst.bass_scheduled_proc)
                    if lane is not None:
                        inst.queue_num = lane % _nq

    nc.compile()
    return nc


# ---------------------------------------------------------------------------
# Runner (PJRT shard_map, compile once)
# ---------------------------------------------------------------------------

class _Runner:
    def __init__(self, nc):
        import jax
        import concourse.mybir as mybir
        from concourse import bass2jax
        from concourse.bass2jax import _bass_exec_p, install_neuronx_cc_hook
        from jax.sharding import Mesh, PartitionSpec
        try:
            from jax.experimental.shard_map import shard_map
        except ImportError:
            from jax.sharding import shard_map
        install_neuronx_cc_hook()
        self.jax = jax
        partition_name = (nc.partition_id_tensor.name
                          if nc.partition_id_tensor else None)
        in_names, out_names, out_avals, zero_outs = [], [], [], []
        for alloc in nc.m.functions[0].allocations:
            if not isinstance(alloc, mybir.MemoryLocationSet):
                continue
            name = alloc.memorylocations[0].name
            if alloc.kind == "ExternalInput":
                if name != partition_name:
                    in_names.append(name)
            elif alloc.kind == "ExternalOutput":
                out_names.append(name)
                shape = tuple(alloc.tensor_shape)
                dtype = mybir.dt.np(alloc.dtype)
                out_avals.append(jax.core.ShapedArray(shape, dtype))
                zero_outs.append(np.zeros(shape, dtype))
        self.in_names, self.out_names = in_names, out_names
        self.out_avals, self.zero_outs = out_avals, zero_outs
        n_params, n_outs = len(in_names), len(out_avals)
        all_in = list(in_names) + list(out_names)
        if partition_name is not None:
            all_in.append(partition_name)

        def _body(*args):
            operands = list(args)
            if partition_name is not None:
                operands.append(bass2jax.partition_id_tensor())
            return tuple(_bass_exec_p.bind(
                *operands, out_avals=tuple(out_avals),
                in_names=tuple(all_in), out_names=tuple(out_names),
                lowering_input_output_aliases=(),
                sim_require_finite=True, sim_require_nnan=True, nc=nc))

        devices = jax.devices()[:N_CORES]
        self.mesh = Mesh(np.asarray(devices), ("core",))
        specs_in = (PartitionSpec("core"),) * (n_params + n_outs)
        specs_out = (PartitionSpec("core"),) * len(out_names)
        self.fn = jax.jit(
            shard_map(_body, mesh=self.mesh, in_specs=specs_in,
                      out_specs=specs_out, check_rep=False),
            keep_unused=True)

    def run(self, in_maps):
        jax = self.jax
        from jax.sharding import NamedSharding, PartitionSpec
        per_core = [[np.asarray(m[n]) for n in self.in_names]
                    for m in in_maps]
        concat = [np.concatenate([per_core[c][i] for c in range(N_CORES)], 0)
                  for i in range(len(self.in_names))]
        zeros = [np.zeros((N_CORES * z.shape[0], *z.shape[1:]), z.dtype)
                 for z in self.zero_outs]
        sh = NamedSharding(self.mesh, PartitionSpec("core"))
        args = [jax.device_put(a, sh) for a in concat + zeros]
        outs = self.fn(*args)
        jax.block_until_ready(outs)
        return [
            {n: np.asarray(outs[i]).reshape(N_CORES,
                                            *self.out_avals[i].shape)[c]
             for i, n in enumerate(self.out_names)}
            for c in range(N_CORES)
        ], (args, outs)


_CACHE = {}


def kernel(**inputs) -> np.ndarray:
    edge_index = np.asarray(inputs["edge_index"])
    x = np.asarray(inputs["x"], dtype=np.float32)

    meta, core_blobs = _prep_edges(edge_index)
    w = _prep_weights(inputs)

    key = "k"
    if key not in _CACHE:
        nc = _build(meta)
        _CACHE[key] = (_Runner(nc), meta)
    runner, _ = _CACHE[key]

    in_maps = _make_in_maps(x, meta, core_blobs, w)
    results, _ = runner.run(in_maps)
    logits = _unpermute(results, meta)
    return (logits, logits)
